# revision 2
# baseline (speedup 1.0000x reference)
"""MinGRU cell kernel for Trainium2 (8 NeuronCores, data-parallel over batch).

Math per sample:
    zh = x @ W.T + b ; z, u = split(zh)
    s = sigmoid(z+bz); a = 1-s
    g = relu(u+bh) + min(sigmoid(u+bh), 0.5)
    h_t = a_t h_{t-1} + s_t g_t          (hardware scan along L)

v2 design vs the original baseline:
  - bf16 matmuls (full-rate like fp32r, but LDWEIGHTS can stream fast and
    x/W DMA+SBUF footprint halves; output error ~2e-3 rel, tolerance 2e-2).
    fp8 DoubleRow was tried and rejected: e4m3's ~2.6%/element quantization
    alone puts the output at 2.4-3.3e-2 rel.
  - weight-resident loop order: x and both W halves live in SBUF; loop is
    4 L-pairs x 8 c-chunks x {2 halves x 8 K-chunks x 2 l-tiles}. Weights are
    reused across the 2 l-tiles of a pair so LDWEIGHTS stays off the
    critical path.
  - epilogue uses only measured-fast ALU shapes (several ALU op pairs run
    ~10x slow on DVE/Pool: stt(SUB,MULT), Pool ts(MIN,BYPASS)...):
      ACT: a = sigmoid(-z-bz), t = sigmoid(u+bh), r = relu(u+bh)
      DVE: g = (t min 0.5) add r; am1 = a*1 + (-1); scan
      Pool: nbv = am1 * g  (= -s*g; scan op1=subtract adds s*g back)
  - h scan output stays f32 (scan state is f32 internally regardless).
"""

import sys
import numpy as np

if "/opt/trn_rl_repo" not in sys.path:
    sys.path.insert(0, "/opt/trn_rl_repo")

from contextlib import ExitStack

import ml_dtypes
import concourse.bass as bass
import concourse.mybir as mybir
import concourse.tile as tile
from concourse import bass_utils
from concourse.bass_utils import run_bass_kernel_spmd

P = 128
N_CORES = 8
L = 4096
H = 1024
HIN = 1024
KC = HIN // P      # 128-row contraction chunks (8)
HC = H // P        # hidden chunks per half (8)
LT = 512           # L columns per matmul / psum bank
NP = 4             # number of L pairs (2*LT each)

F32 = mybir.dt.float32
BF16 = mybir.dt.bfloat16
BFNP = ml_dtypes.bfloat16
AF = mybir.ActivationFunctionType
OP = mybir.AluOpType


def split_waits(nc, max_waits=1):
    """This walrus build only supports one sync wait per instruction; move
    extras onto preceding no-ops on the same engine."""
    for func in nc.m.functions:
        for b in func.blocks:
            idx = 0
            while idx < len(b.instructions):
                inst = b.instructions[idx]
                si = inst.sync_info
                if si is not None and len(si.on_wait) > max_waits:
                    waits = list(si.on_wait)
                    pre, keep = waits[:-max_waits], waits[-max_waits:]
                    pos = idx
                    while pre:
                        chunk, pre = pre[:max_waits], pre[max_waits:]
                        nop = mybir.InstNoOp(
                            name=nc.get_next_instruction_name(), ins=[], outs=[])
                        nop.engine = inst.engine
                        nop.sync_info = mybir.SyncInfo(on_wait=chunk, on_update=[])
                        nc.register_instruction(nop)
                        b.instructions.insert(pos, nop)
                        pos += 1
                        idx += 1
                    si.on_wait = keep
                idx += 1


def build_program():
    nc = bass.Bass()
    xb = nc.dram_tensor("xb", [P, KC, L], BF16, kind="ExternalInput")
    wzb = nc.dram_tensor("wzb", [P, HC, KC, P], BF16, kind="ExternalInput")
    wub = nc.dram_tensor("wub", [P, HC, KC, P], BF16, kind="ExternalInput")
    # aux columns: [-bz | bh | h0], each HC wide
    aux = nc.dram_tensor("aux", [P, 3 * HC], F32, kind="ExternalInput")
    ht = nc.dram_tensor("ht", [P, HC, L], F32, kind="ExternalOutput")

    with tile.TileContext(nc) as tc:
        with ExitStack() as ctx:
            pool = lambda name, bufs: ctx.enter_context(
                tc.tile_pool(name=name, bufs=bufs))
            dat = pool("dat", 1)
            a_pool = pool("a", 3)
            t_pool = pool("t", 3)
            r_pool = pool("r", 3)
            g_pool = pool("g", 3)
            am_pool = pool("am", 3)
            bv_pool = pool("bv", 3)
            h_pool = pool("h", 2)
            psum = ctx.enter_context(
                tc.tile_pool(name="psum", bufs=1, space="PSUM"))

            # ---- warm up the PE clock (HAM gate releases after ~3.4us of
            # activity; these dummy matmuls on garbage SBUF also ride through
            # the cold 1.2 GHz window while input DMAs are in flight) ----
            # ---- DMA schedule: first-needed first ----
            # first L-tile of x alone (smallest possible first dependency),
            # then c=0 slabs of W, then the rest.
            x_t0 = dat.tile([P, KC, LT], BF16, name="x_t0")
            nc.sync.dma_start(x_t0[:], xb[:, :, 0:LT])
            wzA = dat.tile([P, 1, KC, P], BF16, name="wzA")
            nc.sync.dma_start(wzA[:], wzb[:, 0:1])
            wuA = dat.tile([P, 1, KC, P], BF16, name="wuA")
            nc.sync.dma_start(wuA[:], wub[:, 0:1])
            aux_sb = dat.tile([P, 3 * HC], F32)
            nc.sync.dma_start(aux_sb[:], aux[:])
            x_t1 = dat.tile([P, KC, LT], BF16, name="x_t1")
            nc.sync.dma_start(x_t1[:], xb[:, :, LT:2 * LT])
            wzB = dat.tile([P, HC - 1, KC, P], BF16, name="wzB")
            nc.sync.dma_start(wzB[:], wzb[:, 1:HC])
            wuB = dat.tile([P, HC - 1, KC, P], BF16, name="wuB")
            nc.sync.dma_start(wuB[:], wub[:, 1:HC])
            xp = [(x_t0, x_t1)]
            for j in range(1, NP):
                xpj = dat.tile([P, KC, 2 * LT], BF16, name=f"xp{j}")
                nc.sync.dma_start(xpj[:], xb[:, :, j * 2 * LT:(j + 1) * 2 * LT])
                xp.append((xpj[:, :, 0:LT], xpj[:, :, LT:2 * LT]))

            def wslice(half, c, ko):
                wa, wb = (wzA, wzB) if half == 0 else (wuA, wuB)
                if c == 0:
                    return wa[:, 0, ko, :]
                return wb[:, c - 1, ko, :]

            bzN = aux_sb[:, 0 * HC:1 * HC]   # -bz
            bh = aux_sb[:, 1 * HC:2 * HC]
            h0 = aux_sb[:, 2 * HC:3 * HC]

            h_last = [None] * HC

            for j in range(NP):
                for c in range(HC):
                    bk = (j * HC + c) % 2
                    zps = [psum.tile([P, LT], F32, tag=f"z{t}{bk}",
                                     name=f"z{t}{bk}") for t in range(2)]
                    ups = [psum.tile([P, LT], F32, tag=f"u{t}{bk}",
                                     name=f"u{t}{bk}") for t in range(2)]
                    for half, ps in ((0, zps), (1, ups)):
                        for ko in range(KC):
                            w_ap = wslice(half, c, ko)
                            for t in range(2):
                                nc.tensor.matmul(
                                    ps[t][:], w_ap,
                                    xp[j][t][:, ko, :],
                                    start=(ko == 0), stop=(ko == KC - 1))

                    # ---- epilogue for (j, c): two l-tiles ----
                    splits = 1
                    w = LT // splits
                    h_sb = h_pool.tile([P, 2 * LT], F32, tag=f"h{c}",
                                       name=f"h{c}")
                    for t in range(2):
                        for si in range(splits):
                            lo = t * LT + si * w
                            sl = slice(lo, lo + w)
                            psl = slice(si * w, (si + 1) * w)
                            # a = sigmoid(-(z+bz)) = 1 - s            [ACT]
                            a_sb = a_pool.tile([P, w], F32, tag="a",
                                               name="a")
                            nc.scalar.activation(
                                a_sb[:], zps[t][:, psl], AF.Sigmoid,
                                bias=bzN[:, c:c + 1], scale=-1.0)
                            # t = sigmoid(u+bh)                       [ACT]
                            t_sb = t_pool.tile([P, w], F32, tag="t",
                                               name="t")
                            nc.scalar.activation(
                                t_sb[:], ups[t][:, psl], AF.Sigmoid,
                                bias=bh[:, c:c + 1], scale=1.0)
                            # r = relu(u+bh)                          [ACT]
                            r_sb = r_pool.tile([P, w], F32, tag="r",
                                               name="r")
                            nc.scalar.activation(
                                r_sb[:], ups[t][:, psl], AF.Relu,
                                bias=bh[:, c:c + 1], scale=1.0)
                            # g = min(t, 0.5) + r                     [DVE]
                            g_sb = g_pool.tile([P, w], F32, tag="g",
                                               name="g")
                            nc.vector.scalar_tensor_tensor(
                                g_sb[:], t_sb[:], 0.5, r_sb[:],
                                OP.min, OP.add)
                            # om1 = 1 - a = s (as a*-1 + 1)           [DVE]
                            am_sb = am_pool.tile([P, w], F32, tag="am",
                                                 name="am")
                            nc.vector.tensor_scalar(
                                am_sb[:], a_sb[:], -1.0, 1.0,
                                OP.mult, OP.add)
                            # bv = s*g                                [Pool]
                            bv_sb = bv_pool.tile([P, w], F32, tag="bv",
                                                 name="bv")
                            nc.gpsimd.tensor_tensor(
                                bv_sb[:], am_sb[:], g_sb[:], OP.mult)
                            # scan: h = a*h + bv                      [DVE]
                            if t == 0 and si == 0:
                                init = (h0[:, c:c + 1] if j == 0
                                        else h_last[c])
                            else:
                                init = h_sb[:, lo - 1:lo]
                            nc.vector.tensor_tensor_scan(
                                h_sb[:, sl], a_sb[:], bv_sb[:], init,
                                OP.mult, OP.add)
                            if splits > 1:
                                # drain each slice as soon as it's scanned;
                                # separate dispatches land on separate DMA
                                # queues (one queue alone moves only
                                # ~21 GB/s) so the final drain parallelizes
                                nc.sync.dma_start(
                                    ht[:, c, (j * 2 + t) * LT + si * w:
                                       (j * 2 + t) * LT + (si + 1) * w],
                                    h_sb[:, sl])
                    h_last[c] = h_sb[:, 2 * LT - 1:2 * LT]
                    if splits == 1:
                        nc.sync.dma_start(
                            ht[:, c, j * 2 * LT:(j + 1) * 2 * LT], h_sb[:])

    split_waits(nc)
    return nc


_program_cache = {}


def _get_program():
    if "nc" not in _program_cache:
        _program_cache["nc"] = build_program()
    return _program_cache["nc"]


def make_inputs(x, W, b, hx):
    """Host-side packing of full inputs into per-core input maps."""
    x = np.ascontiguousarray(x, dtype=np.float32)
    W = np.ascontiguousarray(W, dtype=np.float32)
    b = np.ascontiguousarray(b, dtype=np.float32)
    hx = np.ascontiguousarray(hx, dtype=np.float32)

    def pack_w(Wh):
        A = Wh.T  # [HIN, H]
        A = A.reshape(KC, P, HC, P).transpose(1, 2, 0, 3)  # [P, HC, KC, P]
        return np.ascontiguousarray(A).astype(BFNP)

    wzb = pack_w(W[:H])
    wub = pack_w(W[H:])
    aux = np.empty((P, 3 * HC), np.float32)
    aux[:, 0:HC] = -b[:H].reshape(HC, P).T
    aux[:, HC:2 * HC] = b[H:].reshape(HC, P).T
    in_maps = []
    for n in range(N_CORES):
        xs = x[n].T.reshape(KC, P, L).transpose(1, 0, 2)  # [P, KC, L]
        xbn = np.ascontiguousarray(xs).astype(BFNP)
        auxn = aux.copy()
        auxn[:, 2 * HC:3 * HC] = hx[n].reshape(HC, P).T
        in_maps.append({"xb": xbn, "wzb": wzb, "wub": wub, "aux": auxn})
    return in_maps


def unpack_output(res):
    out = np.empty((N_CORES, L, H), np.float32)
    for n in range(N_CORES):
        htn = res.results[n]["ht"]  # [P, HC, L]
        out[n] = htn.transpose(2, 1, 0).reshape(L, H)
    return out


def kernel(x, W, b, hx, _debug_result=None):
    assert x.shape == (N_CORES, L, HIN) and W.shape == (2 * H, HIN)
    nc = _get_program()
    in_maps = make_inputs(x, W, b, hx)
    res = run_bass_kernel_spmd(nc, in_maps, core_ids=list(range(N_CORES)))
    if _debug_result is not None:
        _debug_result.append(res)
    return unpack_output(res)


if __name__ == "__main__":
    rng = np.random.default_rng(0)
    x = rng.standard_normal((N_CORES, L, HIN), dtype=np.float32)
    W = (rng.standard_normal((2 * H, HIN), dtype=np.float32) /
         np.sqrt(HIN)).astype(np.float32)
    b = (rng.standard_normal(2 * H) * 0.01).astype(np.float32)
    hx = rng.random((N_CORES, H), dtype=np.float32)
    out = kernel(x, W, b, hx)
    zh = np.einsum("nli,oi->nlo", x, W) + b
    z, u = zh[..., :H], zh[..., H:]
    s = 1 / (1 + np.exp(-z))
    g = np.maximum(u, 0) + np.minimum(1 / (1 + np.exp(-u)), 0.5)
    h = np.empty_like(g)
    prev = hx
    for tt in range(L):
        prev = (1 - s[:, tt]) * prev + s[:, tt] * g[:, tt]
        h[:, tt] = prev
    err = np.abs(out - h).max() / np.abs(h).max()
    print("ran ok", out.shape, "rel err vs linear ref: %.3e" % err)


# revision 3
# speedup vs baseline: 1.0061x; 1.0061x over previous
"""MinGRU cell kernel for Trainium2 (8 NeuronCores, data-parallel over batch).

Math per sample:
    zh = x @ W.T + b ; z, u = split(zh)
    s = sigmoid(z+bz); a = 1-s
    g = relu(u+bh) + min(sigmoid(u+bh), 0.5)
    h_t = a_t h_{t-1} + s_t g_t          (hardware scan along L)

v2 design vs the original baseline:
  - bf16 matmuls (full-rate like fp32r, but LDWEIGHTS can stream fast and
    x/W DMA+SBUF footprint halves; output error ~2e-3 rel, tolerance 2e-2).
    fp8 DoubleRow was tried and rejected: e4m3's ~2.6%/element quantization
    alone puts the output at 2.4-3.3e-2 rel.
  - weight-resident loop order: x and both W halves live in SBUF; loop is
    4 L-pairs x 8 c-chunks x {2 halves x 8 K-chunks x 2 l-tiles}. Weights are
    reused across the 2 l-tiles of a pair so LDWEIGHTS stays off the
    critical path.
  - epilogue uses only measured-fast ALU shapes (several ALU op pairs run
    ~10x slow on DVE/Pool: stt(SUB,MULT), Pool ts(MIN,BYPASS)...):
      ACT: a = sigmoid(-z-bz), t = sigmoid(u+bh), r = relu(u+bh)
      DVE: g = (t min 0.5) add r; am1 = a*1 + (-1); scan
      Pool: nbv = am1 * g  (= -s*g; scan op1=subtract adds s*g back)
  - h scan output stays f32 (scan state is f32 internally regardless).
"""

import sys
import numpy as np

if "/opt/trn_rl_repo" not in sys.path:
    sys.path.insert(0, "/opt/trn_rl_repo")

from contextlib import ExitStack

import ml_dtypes
import concourse.bass as bass
import concourse.mybir as mybir
import concourse.tile as tile
from concourse import bass_utils
from concourse.bass_utils import run_bass_kernel_spmd

P = 128
N_CORES = 8
L = 4096
H = 1024
HIN = 1024
KC = HIN // P      # 128-row contraction chunks (8)
HC = H // P        # hidden chunks per half (8)
LT = 512           # L columns per matmul / psum bank
NP = 4             # number of L pairs (2*LT each)

F32 = mybir.dt.float32
BF16 = mybir.dt.bfloat16
BFNP = ml_dtypes.bfloat16
AF = mybir.ActivationFunctionType
OP = mybir.AluOpType


def split_waits(nc, max_waits=1):
    """This walrus build only supports one sync wait per instruction; move
    extras onto preceding no-ops on the same engine."""
    for func in nc.m.functions:
        for b in func.blocks:
            idx = 0
            while idx < len(b.instructions):
                inst = b.instructions[idx]
                si = inst.sync_info
                if si is not None and len(si.on_wait) > max_waits:
                    waits = list(si.on_wait)
                    pre, keep = waits[:-max_waits], waits[-max_waits:]
                    pos = idx
                    while pre:
                        chunk, pre = pre[:max_waits], pre[max_waits:]
                        nop = mybir.InstNoOp(
                            name=nc.get_next_instruction_name(), ins=[], outs=[])
                        nop.engine = inst.engine
                        nop.sync_info = mybir.SyncInfo(on_wait=chunk, on_update=[])
                        nc.register_instruction(nop)
                        b.instructions.insert(pos, nop)
                        pos += 1
                        idx += 1
                    si.on_wait = keep
                idx += 1


def build_program():
    nc = bass.Bass()
    xb = nc.dram_tensor("xb", [P, KC, L], BF16, kind="ExternalInput")
    wzb = nc.dram_tensor("wzb", [P, HC, KC, P], BF16, kind="ExternalInput")
    wub = nc.dram_tensor("wub", [P, HC, KC, P], BF16, kind="ExternalInput")
    # aux columns: [-bz | bh | h0], each HC wide
    aux = nc.dram_tensor("aux", [P, 3 * HC], F32, kind="ExternalInput")
    # bf16 output: halves the output drain; host upcasts. Adds <=0.4% of h
    # (~4e-3 rel) on top of ~2.3e-3 -- well under the 2e-2 gate.
    ht = nc.dram_tensor("ht", [P, HC, L], BF16, kind="ExternalOutput")

    with tile.TileContext(nc) as tc:
        with ExitStack() as ctx:
            pool = lambda name, bufs: ctx.enter_context(
                tc.tile_pool(name=name, bufs=bufs))
            dat = pool("dat", 1)
            a_pool = pool("a", 3)
            t_pool = pool("t", 3)
            r_pool = pool("r", 3)
            g_pool = pool("g", 3)
            am_pool = pool("am", 3)
            bv_pool = pool("bv", 3)
            h_pool = pool("h", 1)
            psum = ctx.enter_context(
                tc.tile_pool(name="psum", bufs=1, space="PSUM"))

            # ---- warm up the PE clock (HAM gate releases after ~3.4us of
            # activity; these dummy matmuls on garbage SBUF also ride through
            # the cold 1.2 GHz window while input DMAs are in flight) ----
            # ---- DMA schedule: first-needed first ----
            # first L-tile of x in two ko-halves (the first 4 matmuls only
            # need ko 0-3), c=0 slab of Wz in two halves, then the rest.
            x_t0a = dat.tile([P, KC // 2, LT], BF16, name="x_t0a")
            nc.sync.dma_start(x_t0a[:], xb[:, 0:KC // 2, 0:LT])
            wzA1 = dat.tile([P, 1, KC // 2, P], BF16, name="wzA1")
            nc.sync.dma_start(wzA1[:], wzb[:, 0:1, 0:KC // 2])
            x_t0b = dat.tile([P, KC // 2, LT], BF16, name="x_t0b")
            nc.sync.dma_start(x_t0b[:], xb[:, KC // 2:KC, 0:LT])
            wzA2 = dat.tile([P, 1, KC // 2, P], BF16, name="wzA2")
            nc.sync.dma_start(wzA2[:], wzb[:, 0:1, KC // 2:KC])
            wuA = dat.tile([P, 1, KC, P], BF16, name="wuA")
            nc.sync.dma_start(wuA[:], wub[:, 0:1])
            aux_sb = dat.tile([P, 3 * HC], F32)
            nc.sync.dma_start(aux_sb[:], aux[:])
            x_t1 = dat.tile([P, KC, LT], BF16, name="x_t1")
            nc.sync.dma_start(x_t1[:], xb[:, :, LT:2 * LT])
            wzB = dat.tile([P, HC - 1, KC, P], BF16, name="wzB")
            nc.sync.dma_start(wzB[:], wzb[:, 1:HC])
            wuB = dat.tile([P, HC - 1, KC, P], BF16, name="wuB")
            nc.sync.dma_start(wuB[:], wub[:, 1:HC])
            xp = [None]
            for j in range(1, NP):
                xpj = dat.tile([P, KC, 2 * LT], BF16, name=f"xp{j}")
                nc.sync.dma_start(xpj[:], xb[:, :, j * 2 * LT:(j + 1) * 2 * LT])
                xp.append((xpj[:, :, 0:LT], xpj[:, :, LT:2 * LT]))

            def xslice(j, t, ko):
                if j == 0:
                    if t == 0:
                        if ko < KC // 2:
                            return x_t0a[:, ko, :]
                        return x_t0b[:, ko - KC // 2, :]
                    return x_t1[:, ko, :]
                return xp[j][t][:, ko, :]

            def wslice(half, c, ko):
                if half == 0 and c == 0:
                    if ko < KC // 2:
                        return wzA1[:, 0, ko, :]
                    return wzA2[:, 0, ko - KC // 2, :]
                wa, wb = (None, wzB) if half == 0 else (wuA, wuB)
                if c == 0:
                    return wa[:, 0, ko, :]
                return wb[:, c - 1, ko, :]

            bzN = aux_sb[:, 0 * HC:1 * HC]   # -bz
            bh = aux_sb[:, 1 * HC:2 * HC]
            h0 = aux_sb[:, 2 * HC:3 * HC]

            h_last = [None] * HC
            h_tiles = [None] * HC

            for j in range(NP):
                for c in range(HC):
                    bk = (j * HC + c) % 2
                    zps = [psum.tile([P, LT], F32, tag=f"z{t}{bk}",
                                     name=f"z{t}{bk}") for t in range(2)]
                    ups = [psum.tile([P, LT], F32, tag=f"u{t}{bk}",
                                     name=f"u{t}{bk}") for t in range(2)]
                    for half, ps in ((0, zps), (1, ups)):
                        for ko in range(KC):
                            w_ap = wslice(half, c, ko)
                            for t in range(2):
                                nc.tensor.matmul(
                                    ps[t][:], w_ap,
                                    xslice(j, t, ko),
                                    start=(ko == 0), stop=(ko == KC - 1))

                    # ---- epilogue for (j, c): two l-tiles ----
                    splits = 1
                    w = LT // splits
                    # one h tile per c spans ALL of L: scans whose init AP
                    # points into a different tile measure 2188 ns vs 1284
                    # same-tile, so keep the whole chain in one tile; drain
                    # in 2-pair chunks so output DMA still overlaps compute
                    if j == 0:
                        h_tiles[c] = h_pool.tile(
                            [P, L], BF16, tag=f"h{c}", name=f"h{c}")
                    h_sb = h_tiles[c][:, j * 2 * LT:(j + 1) * 2 * LT]
                    for t in range(2):
                        for si in range(splits):
                            lo = t * LT + si * w
                            sl = slice(lo, lo + w)
                            psl = slice(si * w, (si + 1) * w)
                            # a = sigmoid(-(z+bz)) = 1 - s            [ACT]
                            a_sb = a_pool.tile([P, w], F32, tag="a",
                                               name="a")
                            nc.scalar.activation(
                                a_sb[:], zps[t][:, psl], AF.Sigmoid,
                                bias=bzN[:, c:c + 1], scale=-1.0)
                            # t = sigmoid(u+bh)                       [ACT]
                            t_sb = t_pool.tile([P, w], F32, tag="t",
                                               name="t")
                            nc.scalar.activation(
                                t_sb[:], ups[t][:, psl], AF.Sigmoid,
                                bias=bh[:, c:c + 1], scale=1.0)
                            # r = relu(u+bh)                          [ACT]
                            r_sb = r_pool.tile([P, w], F32, tag="r",
                                               name="r")
                            nc.scalar.activation(
                                r_sb[:], ups[t][:, psl], AF.Relu,
                                bias=bh[:, c:c + 1], scale=1.0)
                            # g = min(t, 0.5) + r                     [DVE]
                            g_sb = g_pool.tile([P, w], F32, tag="g",
                                               name="g")
                            nc.vector.scalar_tensor_tensor(
                                g_sb[:], t_sb[:], 0.5, r_sb[:],
                                OP.min, OP.add)
                            # om1 = 1 - a = s (as a*-1 + 1)           [DVE]
                            am_sb = am_pool.tile([P, w], F32, tag="am",
                                                 name="am")
                            nc.vector.tensor_scalar(
                                am_sb[:], a_sb[:], -1.0, 1.0,
                                OP.mult, OP.add)
                            # bv = s*g                                [Pool]
                            bv_sb = bv_pool.tile([P, w], F32, tag="bv",
                                                 name="bv")
                            nc.gpsimd.tensor_tensor(
                                bv_sb[:], am_sb[:], g_sb[:], OP.mult)
                            # scan: h = a*h + bv                      [DVE]
                            if t == 0 and si == 0:
                                init = (h0[:, c:c + 1] if j == 0
                                        else h_last[c])
                            else:
                                init = h_sb[:, lo - 1:lo]
                            nc.vector.tensor_tensor_scan(
                                h_sb[:, sl], a_sb[:], bv_sb[:], init,
                                OP.mult, OP.add)
                            if splits > 1:
                                # drain each slice as soon as it's scanned;
                                # separate dispatches land on separate DMA
                                # queues (one queue alone moves only
                                # ~21 GB/s) so the final drain parallelizes
                                nc.sync.dma_start(
                                    ht[:, c, (j * 2 + t) * LT + si * w:
                                       (j * 2 + t) * LT + (si + 1) * w],
                                    h_sb[:, sl])
                    h_last[c] = h_sb[:, 2 * LT - 1:2 * LT]
                    if j % 2 == 1:
                        nc.sync.dma_start(
                            ht[:, c, (j - 1) * 2 * LT:(j + 1) * 2 * LT],
                            h_tiles[c][:, (j - 1) * 2 * LT:(j + 1) * 2 * LT])

    split_waits(nc)
    return nc


_program_cache = {}


def _get_program():
    if "nc" not in _program_cache:
        _program_cache["nc"] = build_program()
    return _program_cache["nc"]


def make_inputs(x, W, b, hx):
    """Host-side packing of full inputs into per-core input maps."""
    x = np.ascontiguousarray(x, dtype=np.float32)
    W = np.ascontiguousarray(W, dtype=np.float32)
    b = np.ascontiguousarray(b, dtype=np.float32)
    hx = np.ascontiguousarray(hx, dtype=np.float32)

    def pack_w(Wh):
        A = Wh.T  # [HIN, H]
        A = A.reshape(KC, P, HC, P).transpose(1, 2, 0, 3)  # [P, HC, KC, P]
        return np.ascontiguousarray(A).astype(BFNP)

    wzb = pack_w(W[:H])
    wub = pack_w(W[H:])
    aux = np.empty((P, 3 * HC), np.float32)
    aux[:, 0:HC] = -b[:H].reshape(HC, P).T
    aux[:, HC:2 * HC] = b[H:].reshape(HC, P).T
    in_maps = []
    for n in range(N_CORES):
        xs = x[n].T.reshape(KC, P, L).transpose(1, 0, 2)  # [P, KC, L]
        xbn = np.ascontiguousarray(xs).astype(BFNP)
        auxn = aux.copy()
        auxn[:, 2 * HC:3 * HC] = hx[n].reshape(HC, P).T
        in_maps.append({"xb": xbn, "wzb": wzb, "wub": wub, "aux": auxn})
    return in_maps


def unpack_output(res):
    out = np.empty((N_CORES, L, H), np.float32)
    for n in range(N_CORES):
        htn = np.asarray(res.results[n]["ht"]).astype(np.float32)
        out[n] = htn.transpose(2, 1, 0).reshape(L, H)  # [P, HC, L] -> [L, H]
    return out


def kernel(x, W, b, hx, _debug_result=None):
    assert x.shape == (N_CORES, L, HIN) and W.shape == (2 * H, HIN)
    nc = _get_program()
    in_maps = make_inputs(x, W, b, hx)
    res = run_bass_kernel_spmd(nc, in_maps, core_ids=list(range(N_CORES)))
    if _debug_result is not None:
        _debug_result.append(res)
    return unpack_output(res)


if __name__ == "__main__":
    rng = np.random.default_rng(0)
    x = rng.standard_normal((N_CORES, L, HIN), dtype=np.float32)
    W = (rng.standard_normal((2 * H, HIN), dtype=np.float32) /
         np.sqrt(HIN)).astype(np.float32)
    b = (rng.standard_normal(2 * H) * 0.01).astype(np.float32)
    hx = rng.random((N_CORES, H), dtype=np.float32)
    out = kernel(x, W, b, hx)
    zh = np.einsum("nli,oi->nlo", x, W) + b
    z, u = zh[..., :H], zh[..., H:]
    s = 1 / (1 + np.exp(-z))
    g = np.maximum(u, 0) + np.minimum(1 / (1 + np.exp(-u)), 0.5)
    h = np.empty_like(g)
    prev = hx
    for tt in range(L):
        prev = (1 - s[:, tt]) * prev + s[:, tt] * g[:, tt]
        h[:, tt] = prev
    err = np.abs(out - h).max() / np.abs(h).max()
    print("ran ok", out.shape, "rel err vs linear ref: %.3e" % err)
